# revision 2
# baseline (speedup 1.0000x reference)
"""DiffWave forward pass on 8 Trainium2 NeuronCores (Bass/Tile) — v2.

Sharding: core c -> (batch b = c//2, half h = c%2). Odd cores get time-REVERSED
audio + tap-reversed conv weights, so every core's margin is on the RIGHT and
the per-layer compute window can shrink with the remaining receptive field
(11 -> 8 chunks over the 30 layers). Margins are one-sided: the true sequence
edge is always at the LEFT (beffL edge-bias correction only).

Per-core resident state (bf16):
  xs  [128, WBUF]: rows 0-63 trunk x_l (deferred 2^{-l/2} scale folded into
      weights), rows 64-127 h-accumulator = tail-folded skip head
      (h = sum_l (sk_W/sqrt(30) @ op_W_sk[l]) @ mt_l).
  xsP [128, WBUF]: rows 0-63 trunk copy (GPSIMD bitcast-int32 copy), rows
      64-127 trunk shifted LEFT by d_l (DMA partition-shift) -> the dilated
      conv runs as 2 K=128 GEMMs: [W1;W2] @ xsP(col) + [W0;0] @ xs(col-d).

Gating: one packed ACT tanh per chunk over 128 partitions -> opin [tg; tf]
(gate prescaled 0.5 in weights; (tg+1)*tf = tg*tf + tf is folded into the
1x1 conv with K=128 duplicated weights). The tg*tf product is computed
in-place on opin rows 0-63, split column-wise between DVE and GPSIMD.
Trunk+h update = one DVE tensor_add per 2-chunk pair (PSUM f32 source).
Chunk sweep direction alternates per layer (boustrophedon) so consecutive
layers pipeline without a whole-layer dependency bubble.
"""

import os
import sys

sys.path.insert(0, "/opt/trn_rl_repo")

import numpy as np

import concourse.bacc as bacc
import concourse.mybir as mybir
import concourse.tile as tile

f32 = mybir.dt.float32
f32r = mybir.dt.float32r
bf16 = mybir.dt.bfloat16
u32 = mybir.dt.uint32
AF = mybir.ActivationFunctionType
ALU = mybir.AluOpType

C = 64
L = 30
B = 4
T = 16384
MAX_STEPS = 200
OWN = T // 2
PAD = 512
E = OWN + 3072
WBUF = PAD + E + PAD   # 12288
CH = 1024
NCH0 = E // CH         # 11
DILS = [2 ** (i % 10) for i in range(L)]
_rsum = np.cumsum(DILS)
NCHL = [int(np.ceil((OWN + 3069 - _rsum[l]) / CH)) for l in range(L)]

GPF = float(os.environ.get("KGPF", "0.32"))   # fraction of mt cols on GPSIMD
TOPDMA = os.environ.get("KTOPDMA", "0") == "1"  # xsP top copy via DMA not GPSIMD
PUP = os.environ.get("KPUP", "0") == "1"        # paired (2-chunk) updates
CPY = os.environ.get("KCPY", "P")               # xsP top-copy engine rotation
HPRI = int(os.environ.get("KHPRI", "150"))      # refresh priority boost
EVF = float(os.environ.get("KEVF", "0.0"))      # fraction of updates via ACT evac

_CACHE = {}


def _pairs(order):
    out = []
    i = 0
    while i < len(order):
        out.append(tuple(order[i : i + 2]))
        i += 2
    return out


# --------------------------------------------------------------------------
# device program
# --------------------------------------------------------------------------
def _build_program(dbg=False):
    nc = bacc.Bacc(
        "TRN2",
        target_bir_lowering=False,
        debug=False,
        enable_asserts=False,
        num_devices=8,
    )

    dram = {}

    def din(name, shape, dtype):
        dram[name] = nc.dram_tensor(name, list(shape), dtype, kind="ExternalInput")
        return dram[name]

    din("aud", [1, E], f32)
    din("w3p", [128, L * 128], bf16)    # GEMM1 lhsT [W1.T; W2.T] per layer
    din("w30", [128, L * 128], bf16)    # GEMM2 lhsT [W0.T; 0] per layer
    din("opw2", [128, L * 128], bf16)   # 1x1 lhsT [Wop.T; Wop.T] per layer
    din("wsum", [128, 15 * 128], bf16)   # beff lhsT, layer pairs stacked
    din("bconst", [128, L], f32)
    din("wtl", [128, 15 * 128], bf16)    # left-edge lhsT (drop tap 0)
    din("bcl", [128, L], f32)
    din("dpw", [128, 15 * 4 * 128], bf16)
    din("dpb", [128, 15], f32)
    din("p1", [128, 512], bf16)
    din("p1b", [128, 4], f32)
    din("p2", [128, 16 * 128], bf16)
    din("p2b", [128, 4], f32)
    din("emb", [128, 1], bf16)
    din("inw", [128, C], f32r)          # row 0 = in_W, rest 0
    din("inb", [C, 1], f32)
    din("skbx", [128, 1], f32)          # rows 64-127 = skb_eff, rows 0-63 = 0
    din("outw", [128, 1], bf16)         # rows 64-127 = out_W, rows 0-63 = 0
    din("outb", [1, 1], f32)
    o_d = nc.dram_tensor("o", [1, OWN], f32, kind="ExternalOutput")
    DBG_LAYERS = (0, 1, 4, 9, 29)
    if dbg:
        dbg_d = {
            "d_beff": nc.dram_tensor("d_beff", [128, L], f32, kind="ExternalOutput"),
            "d_beffL": nc.dram_tensor("d_beffL", [128, L], f32, kind="ExternalOutput"),
            "d_x0": nc.dram_tensor("d_x0", [128, WBUF], f32, kind="ExternalOutput"),
        }
        for dl in DBG_LAYERS:
            dbg_d[f"d_xs{dl}"] = nc.dram_tensor(
                f"d_xs{dl}", [128, WBUF], f32, kind="ExternalOutput"
            )

    with tile.TileContext(nc) as tc:
        import contextlib

        ctx = contextlib.ExitStack()
        with ctx:
            const = ctx.enter_context(tc.tile_pool(name="const", bufs=1))
            opinp = ctx.enter_context(tc.tile_pool(name="opinp", bufs=4))
            sgfp = ctx.enter_context(tc.tile_pool(name="sgfp", bufs=4))
            hhp = ctx.enter_context(tc.tile_pool(name="hhp", bufs=2))
            evp = ctx.enter_context(tc.tile_pool(name="evp", bufs=3))
            otp = ctx.enter_context(tc.tile_pool(name="otp", bufs=2))
            dil_ps = ctx.enter_context(tc.tile_pool(name="dil_ps", bufs=2, space="PSUM"))
            op_ps = ctx.enter_context(tc.tile_pool(name="op_ps", bufs=2, space="PSUM"))

            # ---- resident state + weights ----
            xsA = const.tile([128, WBUF], bf16)   # trunk+h, layers even read
            xsB = const.tile([128, WBUF], bf16)   # trunk+h, layers odd read
            xsP = const.tile([128, WBUF], bf16)
            w3p = const.tile([128, L * 128], bf16)
            w30 = const.tile([128, L * 128], bf16)
            opw2 = const.tile([128, L * 128], bf16)
            bconst = const.tile([128, L], f32)
            beff = const.tile([128, L], f32)
            bcl = const.tile([128, L], f32)
            beffL = const.tile([128, L], f32)
            inw = const.tile([128, C], f32r)
            inb = const.tile([C, 1], f32)
            skbx = const.tile([128, 1], f32)
            outw = const.tile([128, 1], bf16)
            outb = const.tile([1, 1], f32)

            nc.sync.dma_start(bconst[:], dram["bconst"].ap())
            nc.sync.dma_start(bcl[:], dram["bcl"].ap())
            nc.sync.dma_start(inw[:], dram["inw"].ap())
            nc.sync.dma_start(inb[:], dram["inb"].ap())

            # zero-init: left pads (GEMM2 reads them); h rows of the x0-parity
            # buffer (accumulator). xsP pads are never read.
            nc.vector.memset(xsA[:, 0:PAD], 0.0)
            nc.vector.memset(xsB[:, 0:PAD], 0.0)
            nc.gpsimd.memset(xsA[C:128, PAD:WBUF].bitcast(u32), 0)

            with (
                tc.tile_pool(name="pre", bufs=1) as pre,
                tc.tile_pool(name="audp", bufs=2) as audp,
            ):
                # ---- diffusion embedding MLP + cond + beff (tiny, fp32) ----
                dpw = pre.tile([128, 15 * 4 * 128], bf16)
                dpb = pre.tile([128, 15], f32)
                p1 = pre.tile([128, 512], bf16)
                p1b = pre.tile([128, 4], f32)
                p2 = pre.tile([128, 16 * 128], bf16)
                p2b = pre.tile([128, 4], f32)
                wsum = pre.tile([128, 15 * 128], bf16)
                wtl = pre.tile([128, 15 * 128], bf16)
                emb = pre.tile([128, 1], bf16)
                t1 = pre.tile([128, 4], bf16)
                t2 = pre.tile([128, 4], bf16)
                cond = pre.tile([128, 15], bf16)
                nc.sync.dma_start(p1[:], dram["p1"].ap())
                nc.sync.dma_start(p1b[:], dram["p1b"].ap())
                nc.sync.dma_start(p2[:], dram["p2"].ap())
                nc.sync.dma_start(p2b[:], dram["p2b"].ap())
                nc.sync.dma_start(dpw[:], dram["dpw"].ap())
                nc.sync.dma_start(dpb[:], dram["dpb"].ap())
                nc.sync.dma_start(wsum[:], dram["wsum"].ap())
                nc.sync.dma_start(wtl[:], dram["wtl"].ap())
                nc.sync.dma_start(emb[:], dram["emb"].ap())
                HD = 3 * 128
                for wt_t, wt_d in ((w3p, "w3p"), (w30, "w30"), (opw2, "opw2")):
                    nc.sync.dma_start(wt_t[:, 0:HD], dram[wt_d].ap()[:, 0:HD])
                for wt_t, wt_d in ((w3p, "w3p"), (w30, "w30"), (opw2, "opw2")):
                    nc.sync.dma_start(
                        wt_t[:, HD : L * 128], dram[wt_d].ap()[:, HD : L * 128]
                    )
                nc.sync.dma_start(skbx[:], dram["skbx"].ap())
                nc.sync.dma_start(outw[:], dram["outw"].ap())
                nc.sync.dma_start(outb[:], dram["outb"].ap())

                ps_t1 = dil_ps.tile([128, CH], f32, tag="dil")
                for i in range(4):
                    nc.tensor.matmul(
                        ps_t1[:, i : i + 1],
                        lhsT=p1[:, i * 128 : (i + 1) * 128],
                        rhs=emb[:, 0:1],
                        start=True,
                        stop=True,
                    )
                sgv1 = pre.tile([128, 4], f32)
                for i in range(4):
                    nc.scalar.activation(
                        sgv1[:, i : i + 1], ps_t1[:, i : i + 1], AF.Sigmoid,
                        bias=p1b[:, i : i + 1],
                    )
                for i in range(4):
                    nc.vector.scalar_tensor_tensor(
                        t1[:, i : i + 1], ps_t1[:, i : i + 1], p1b[:, i : i + 1],
                        sgv1[:, i : i + 1], ALU.add, ALU.mult,
                    )
                ps_t2 = dil_ps.tile([128, CH], f32, tag="dil")
                for i in range(4):
                    for j in range(4):
                        nc.tensor.matmul(
                            ps_t2[:, i : i + 1],
                            lhsT=p2[:, (i * 4 + j) * 128 : (i * 4 + j + 1) * 128],
                            rhs=t1[:, j : j + 1],
                            start=(j == 0),
                            stop=(j == 3),
                        )
                sgv2 = pre.tile([128, 4], f32)
                for i in range(4):
                    nc.scalar.activation(
                        sgv2[:, i : i + 1], ps_t2[:, i : i + 1], AF.Sigmoid,
                        bias=p2b[:, i : i + 1],
                    )
                for i in range(4):
                    nc.vector.scalar_tensor_tensor(
                        t2[:, i : i + 1], ps_t2[:, i : i + 1], p2b[:, i : i + 1],
                        sgv2[:, i : i + 1], ALU.add, ALU.mult,
                    )
                ps_cond = dil_ps.tile([128, CH], f32, tag="dil")
                for c in range(15):
                    for j in range(4):
                        nc.tensor.matmul(
                            ps_cond[:, c : c + 1],
                            lhsT=dpw[:, (c * 4 + j) * 128 : (c * 4 + j + 1) * 128],
                            rhs=t2[:, j : j + 1],
                            start=(j == 0),
                            stop=(j == 3),
                        )
                nc.vector.tensor_add(cond[:], ps_cond[:, 0:15], dpb[:])
                for wmat, bvec, bout in ((wsum, bconst, beff), (wtl, bcl, beffL)):
                    ps_beff = dil_ps.tile([128, CH], f32, tag="dil", name="ps_beff")
                    for l in range(L):
                        c = l // 2
                        if l % 2 == 0:
                            nc.tensor.matmul(
                                ps_beff[:, l : l + 1],
                                lhsT=wmat[0:C, c * 128 : (c + 1) * 128],
                                rhs=cond[0:C, c : c + 1],
                                start=True,
                                stop=True,
                                tile_position=(0, 0),
                            )
                        else:
                            nc.tensor.matmul(
                                ps_beff[:, l : l + 1],
                                lhsT=wmat[C:128, c * 128 : (c + 1) * 128],
                                rhs=cond[C:128, c : c + 1],
                                start=True,
                                stop=True,
                                tile_position=(64, 0),
                            )
                    nc.vector.tensor_add(bout[:], ps_beff[:, 0:L], bvec[:])

                # ---- input conv: x0 = relu(in_W * audio + in_b), forward ----
                d0 = DILS[0]
                for c in range(NCH0):
                    col = PAD + c * CH
                    at = audp.tile([128, CH], f32r, tag="aud")
                    if c < 2:  # rows 1-127 multiplied by zero weights: keep finite
                        nc.vector.memset(at[:].bitcast(u32), 0)
                    nc.scalar.dma_start(
                        at[0:1, :], dram["aud"].ap()[:, c * CH : (c + 1) * CH].bitcast(f32r)
                    )
                    x0 = dil_ps.tile([128, CH], f32, tag="dil")
                    for s in (0, 512):
                        nc.tensor.matmul(
                            x0[0:C, s : s + 512],
                            lhsT=inw[:],
                            rhs=at[:, s : s + 512],
                            start=True,
                            stop=True,
                        )
                    nc.scalar.activation(
                        xsA[0:C, col : col + CH],
                        x0[0:C, :],
                        AF.Relu,
                        bias=inb[:, 0:1],
                    )
                    if c % 2 == 1 or c == NCH0 - 1:
                        a0 = PAD + (c - (c % 2)) * CH if c % 2 == 1 else col
                        b0 = col + CH
                        nc.gpsimd.tensor_copy(
                            xsP[0:C, a0:b0].bitcast(u32), xsA[0:C, a0:b0].bitcast(u32)
                        )
                        nc.sync.dma_start(
                            xsP[C:128, a0 - d0 : b0 - d0], xsA[0:C, a0:b0]
                        )

                if dbg:
                    nc.sync.dma_start(dbg_d["d_beff"].ap(), beff[:])
                    nc.sync.dma_start(dbg_d["d_beffL"].ap(), beffL[:])
                    dx = pre.tile([128, WBUF], f32)
                    nc.scalar.activation(dx[:], xsA[:], AF.Copy)
                    nc.sync.dma_start(dbg_d["d_x0"].ap(), dx[:])

            # ---- 30 residual layers ----
            cpy_i = 0
            ev_state = [0.0]

            def emit_update(xw_t, xr_t, ppa, pop):
                ev_state[0] += EVF
                if ev_state[0] >= 1.0:
                    ev_state[0] -= 1.0
                    ev = evp.tile([128, CH], bf16, tag="ev")
                    nc.scalar.copy(ev[:], pop[:])
                    nc.vector.tensor_add(
                        xw_t[:, ppa : ppa + CH], xr_t[:, ppa : ppa + CH], ev[:]
                    )
                else:
                    nc.vector.tensor_add(
                        xw_t[:, ppa : ppa + CH], xr_t[:, ppa : ppa + CH], pop[:]
                    )

            for l in range(L):
                xr = xsA if l % 2 == 0 else xsB
                xw = xsB if l % 2 == 0 else xsA
                d = DILS[l]
                nch = NCHL[l]
                d2 = DILS[l + 1] if l + 1 < L else 0
                order = list(range(nch))
                if (l % 2 == 0) and os.environ.get("KBOUS", "0") == "1":
                    order = order[::-1]
                pend = None
                pref = None
                opin = None
                for ci, c in enumerate(order):
                    col = PAD + c * CH
                    half = ci % 2
                    if half == 0:
                        opin = opinp.tile([128, 2 * CH], bf16, tag="opin")
                        pair_base = col
                    dil = dil_ps.tile([128, CH], f32, tag="dil")
                    for s in (0, 512):
                        nc.tensor.matmul(
                            dil[:, s : s + 512],
                            lhsT=w3p[:, l * 128 : (l + 1) * 128],
                            rhs=xsP[:, col + s : col + s + 512],
                            start=True,
                            stop=False,
                        )
                        nc.tensor.matmul(
                            dil[:, s : s + 512],
                            lhsT=w30[:, l * 128 : (l + 1) * 128],
                            rhs=xr[:, col - d + s : col - d + s + 512],
                            start=False,
                            stop=True,
                        )
                    # deferred trunk/h update (one-chunk lag) + xsP refresh
                    if pend is not None:
                        ppa, pop = pend
                        emit_update(xw, xr, ppa, pop)
                        pend = None
                        if pref is not None and d2:
                            ra, rb = pref
                            with tc.high_priority(offset=HPRI):
                                for rc in range(ra, rb, CH):
                                    nc.gpsimd.tensor_copy(
                                        xsP[0:C, rc : rc + CH].bitcast(u32),
                                        xw[0:C, rc : rc + CH].bitcast(u32),
                                    )
                                nc.sync.dma_start(
                                    xsP[C:128, ra - d2 : rb - d2], xw[0:C, ra:rb]
                                )
                            pref = None
                    # tanh into the pair tile (edge bias pieces on chunk 0)
                    off = half * CH
                    if c == 0:
                        pieces = [(0, d, beffL), (d, CH, beff)]
                    else:
                        pieces = [(0, CH, beff)]
                    for lo, hi, bv in pieces:
                        nc.scalar.activation(
                            opin[:, off + lo : off + hi], dil[:, lo:hi], AF.Tanh,
                            bias=bv[:, l : l + 1],
                        )
                    if half == 1 or ci == nch - 1:
                        pcols = (half + 1) * CH
                        sgf = sgfp.tile([C, 2 * CH], bf16, tag="sgf")
                        nc.sync.dma_start(sgf[:, 0:pcols], opin[C:128, 0:pcols])
                        gcols = (int(pcols * GPF) // 64) * 64
                        dcols = pcols - gcols
                        nc.vector.tensor_mul(
                            opin[0:C, 0:dcols], opin[0:C, 0:dcols], sgf[:, 0:dcols]
                        )
                        if gcols:
                            nc.gpsimd.tensor_mul(
                                opin[0:C, dcols:pcols],
                                opin[0:C, dcols:pcols],
                                sgf[:, dcols:pcols],
                            )
                        base = pair_base
                        for hh2 in range(half + 1):
                            op = op_ps.tile([128, CH], f32, tag="op")
                            for s in (0, 512):
                                nc.tensor.matmul(
                                    op[:, s : s + 512],
                                    lhsT=opw2[:, l * 128 : (l + 1) * 128],
                                    rhs=opin[:, hh2 * CH + s : hh2 * CH + s + 512],
                                    start=True,
                                    stop=True,
                                )
                            if hh2 < half:
                                emit_update(xw, xr, base, op)
                            else:
                                pend = (base, op)
                            base += CH
                        pref = (pair_base, pair_base + pcols)
                ppa, pop = pend
                emit_update(xw, xr, ppa, pop)
                if d2:
                    ra, rb = pref
                    with tc.high_priority(offset=HPRI):
                        for rc in range(ra, rb, CH):
                            nc.gpsimd.tensor_copy(
                                xsP[0:C, rc : rc + CH].bitcast(u32),
                                xw[0:C, rc : rc + CH].bitcast(u32),
                            )
                        nc.sync.dma_start(
                            xsP[C:128, ra - d2 : rb - d2], xw[0:C, ra:rb]
                        )
                if dbg and l in DBG_LAYERS:
                    with tc.tile_pool(name=f"dbgp{l}", bufs=1) as dbgp:
                        dx = dbgp.tile([128, WBUF], f32)
                        nc.scalar.activation(dx[:], xw[:], AF.Copy)
                        nc.sync.dma_start(dbg_d[f"d_xs{l}"].ap(), dx[:])

            # ---- tail: hh = relu(h + skb), out = outw.T @ hh + outb ----
            for c in range(OWN // CH - 1, -1, -1):  # reverse (layer 29 was forward)
                col = PAD + c * CH
                hh = hhp.tile([128, CH], bf16, tag="hh")
                nc.scalar.activation(hh[:], xsA[:, col : col + CH], AF.Relu,
                                     bias=skbx[:, 0:1])
                ops2 = dil_ps.tile([128, CH], f32, tag="dil")
                for s in (0, 512):
                    nc.tensor.matmul(
                        ops2[0:1, s : s + 512],
                        lhsT=outw[:],
                        rhs=hh[:, s : s + 512],
                        start=True,
                        stop=True,
                    )
                ot = otp.tile([1, CH], f32, tag="ot")
                nc.scalar.add(ot[:], ops2[0:1, 0:CH], outb[0:1, 0:1])
                nc.sync.dma_start(o_d.ap()[:, c * CH : (c + 1) * CH], ot[:])

    nc.compile()
    return nc


# --------------------------------------------------------------------------
# host-side weight folding
# --------------------------------------------------------------------------
def _emb_table():
    steps = np.arange(MAX_STEPS, dtype=np.float32)[:, None]
    dims = np.arange(64, dtype=np.float32)[None, :]
    t = steps * 10.0 ** (dims * 4.0 / 63.0)
    return np.concatenate([np.sin(t), np.cos(t)], axis=1).astype(np.float32)


def _fold_core(dwp, inputs):
    """Per-tap-order folds. dwp: [L,128,64,3] (tap-reversed for odd cores)."""
    import ml_dtypes

    f = lambda a: np.ascontiguousarray(np.asarray(a), dtype=np.float32)
    dw_b = f(inputs["dw_b"])
    op_W, op_b = f(inputs["op_W"]), f(inputs["op_b"])
    sk_W = f(inputs["sk_W"])

    sc = np.float32(2.0) ** (-np.arange(L, dtype=np.float32) / 2)
    scu = np.float32(2.0) ** (np.arange(L, dtype=np.float32) / 2)
    Sg = np.ones((128, 1), np.float32)
    Sg[0:C] = 0.5

    w3p = np.zeros((128, L * 128), np.float32)
    w30 = np.zeros((128, L * 128), np.float32)
    for l in range(L):
        W1 = (dwp[l, :, :, 1] * sc[l] * Sg).T  # [64in, 128out]
        W2 = (dwp[l, :, :, 2] * sc[l] * Sg).T
        W0 = (dwp[l, :, :, 0] * sc[l] * Sg).T
        w3p[0:C, l * 128 : (l + 1) * 128] = W1
        w3p[C:128, l * 128 : (l + 1) * 128] = W2
        w30[0:C, l * 128 : (l + 1) * 128] = W0

    # beff folds (wsum over all taps; wtl drops tap 0), with omega deferred
    # op-bias corrections
    wsum_raw = dwp.sum(axis=3)
    wtl_raw = dwp[:, :, :, 1:].sum(axis=3)
    wsum = np.zeros((128, 15 * 128), np.float32)
    wtl = np.zeros((128, 15 * 128), np.float32)
    bconst = np.zeros((128, L), np.float32)
    bcl = np.zeros((128, L), np.float32)
    omega = np.zeros(C, np.float32)
    for l in range(L):
        cc = l // 2
        rows = slice(0, C) if l % 2 == 0 else slice(C, 128)
        cols = slice(cc * 128, (cc + 1) * 128)
        wsum[rows, cols] = (wsum_raw[l] * Sg).T
        wtl[rows, cols] = (wtl_raw[l] * Sg).T
        bconst[:, l] = Sg[:, 0] * (dw_b[l] + sc[l] * (wsum_raw[l] @ omega))
        bcl[:, l] = Sg[:, 0] * (dw_b[l] + sc[l] * (wtl_raw[l] @ omega))
        omega = omega + scu[l] * op_b[l, 0:C]

    # 1x1 conv lhsT: cols 0-63 res (x0.5, x2^{l/2}), cols 64-127 h-fold;
    # rows duplicated (input is [tg*tf; tf])
    skw_s = sk_W[:, :, 0] / np.sqrt(np.float32(L))
    opw2 = np.zeros((128, L * 128), np.float32)
    for l in range(L):
        Wres = op_W[l, 0:C, :, 0] * 0.5 * scu[l]       # [64res, 64in]
        Wh = (skw_s @ op_W[l, C:, :, 0]) * 0.5         # [64h, 64in]
        Wop = np.concatenate([Wres, Wh], axis=0).T     # [64in, 128out]
        opw2[0:C, l * 128 : (l + 1) * 128] = Wop
        opw2[C:128, l * 128 : (l + 1) * 128] = Wop

    return {
        "w3p": w3p.astype(ml_dtypes.bfloat16),
        "w30": w30.astype(ml_dtypes.bfloat16),
        "opw2": opw2.astype(ml_dtypes.bfloat16),
        "wsum": wsum.astype(ml_dtypes.bfloat16),
        "bconst": bconst,
        "wtl": wtl.astype(ml_dtypes.bfloat16),
        "bcl": bcl,
    }


def _prep_maps(inputs):
    import ml_dtypes

    f = lambda a: np.ascontiguousarray(np.asarray(a), dtype=np.float32)
    audio = f(inputs["audio"])
    step = np.asarray(inputs["diffusion_step"]).astype(np.int64)
    in_W, in_b = f(inputs["in_W"]), f(inputs["in_b"])
    p1_W, p1_b = f(inputs["p1_W"]), f(inputs["p1_b"])
    p2_W, p2_b = f(inputs["p2_W"]), f(inputs["p2_b"])
    dw_W = f(inputs["dw_W"])
    dp_W, dp_b = f(inputs["dp_W"]), f(inputs["dp_b"])
    op_b = f(inputs["op_b"])
    sk_W, sk_b = f(inputs["sk_W"]), f(inputs["sk_b"])
    out_W, out_b = f(inputs["out_W"]), f(inputs["out_b"])

    # cond lhsT, layer pairs stacked on partitions, 4 k-chunks each
    dpw = np.zeros((128, 15 * 4 * 128), np.float32)
    dpb = np.zeros((128, 15), np.float32)
    for c in range(15):
        for j in range(4):
            blk = np.zeros((128, 128), np.float32)
            blk[:, 0:C] = dp_W[2 * c][:, j * 128 : (j + 1) * 128].T
            blk[:, C:128] = dp_W[2 * c + 1][:, j * 128 : (j + 1) * 128].T
            dpw[:, (c * 4 + j) * 128 : (c * 4 + j + 1) * 128] = blk
        dpb[0:C, c] = dp_b[2 * c]
        dpb[C:128, c] = dp_b[2 * c + 1]

    p1 = p1_W.T.copy()
    p1b = p1_b.reshape(4, 128).T.copy()
    p2 = np.zeros((128, 16 * 128), np.float32)
    p2T = p2_W.T
    for i in range(4):
        for j in range(4):
            p2[:, (i * 4 + j) * 128 : (i * 4 + j + 1) * 128] = p2T[
                j * 128 : (j + 1) * 128, i * 128 : (i + 1) * 128
            ]
    p2b = p2_b.reshape(4, 128).T.copy()

    inw = np.zeros((128, C), np.float32)
    inw[0] = in_W[:, 0, 0]
    skbx = np.zeros((128, 1), np.float32)
    skbx[C:128, 0] = sk_b + sk_W[:, :, 0] @ op_b[:, C:].sum(axis=0) / np.sqrt(
        np.float32(L)
    )
    outw = np.zeros((128, 1), np.float32)
    outw[C:128, 0] = out_W[0, :, 0]

    table = _emb_table()

    shared = {
        "dpw": dpw.astype(ml_dtypes.bfloat16),
        "dpb": dpb,
        "p1": np.ascontiguousarray(p1).astype(ml_dtypes.bfloat16),
        "p1b": np.ascontiguousarray(p1b),
        "p2": p2.astype(ml_dtypes.bfloat16),
        "p2b": np.ascontiguousarray(p2b),
        "inw": inw,
        "inb": in_b.reshape(C, 1),
        "skbx": skbx,
        "outw": outw.astype(ml_dtypes.bfloat16),
        "outb": out_b.reshape(1, 1),
    }
    fold_f = _fold_core(dw_W, inputs)
    fold_r = _fold_core(np.ascontiguousarray(dw_W[:, :, :, ::-1]), inputs)

    in_maps = []
    for core in range(8):
        b, h = core // 2, core % 2
        if h == 0:
            aud = audio[b, 0, 0:E]
            fold = fold_f
        else:
            aud = audio[b, 0, ::-1][0:E]
            fold = fold_r
        m = dict(shared)
        m.update(fold)
        m["aud"] = np.ascontiguousarray(aud.reshape(1, E))
        m["emb"] = np.ascontiguousarray(table[int(step[b])].reshape(128, 1)).astype(ml_dtypes.bfloat16)
        in_maps.append(m)
    return in_maps


def _get_nc():
    if "nc" not in _CACHE:
        _CACHE["nc"] = _build_program()
    return _CACHE["nc"]


def kernel(**inputs) -> np.ndarray:
    from concourse.bass_utils import run_bass_kernel_spmd

    nc = _get_nc()
    in_maps = _prep_maps(inputs)
    res = run_bass_kernel_spmd(nc, in_maps, core_ids=list(range(8))).results
    out = np.zeros((B, 1, T), np.float32)
    for b in range(B):
        out[b, 0, 0:OWN] = res[2 * b]["o"][0, :]
        out[b, 0, OWN:T] = res[2 * b + 1]["o"][0, ::-1]
    return out
